# revision 8
# baseline (speedup 1.0000x reference)
"""HGNN (2-stage hypergraph conv) kernel for Trainium2.

Data-parallel over batch across 8 NeuronCores (16 batches/core). Weights and
biases are embedded in the NEFF as Const tensors (cast to bf16) so they are
DMA'd to HBM once at model-load time; per-iteration traffic is only the
activations (bf16) in and the f32 output out. Matmuls run on the PE in bf16
with f32 PSUM accumulation.

Per-core plan (stage = conv(conv(x))):
  G setup     : G = DV^-1/2 Hs DE^-1 Hs^T DV^-1/2 computed on-device (tiny, fp32).
                G is symmetric. g = G @ 1 for the aggregated-bias term.
  phase A     : A_fm[d,(b,m)] = (G X_b)^T   -- AGG-B: activation-stationary
                matmuls (lhsT=X_b[80,128-dtile], rhs=G) -> RM->FM "free" transpose.
  phase B     : H_fm = relu(A_fm.T W1 + g (x) b1) -- weight-stationary matmuls
                accumulating over din tiles + a K=1 bias-row matmul; ACT relu
                copyback straight from PSUM (FM->FM).
  phase C     : per (dout-chunk, batch): Y = H_b^T W2 + b2 (activation-stationary,
                FM->RM), then Z = G Y (G-stationary), DMA out.
"""
import os
import numpy as np

_CACHE = {}

B_PER_CORE = 16
NN = 80
R = B_PER_CORE * NN  # 1280
N_CORES = 8


def _bf16(a):
    import ml_dtypes
    return np.ascontiguousarray(np.asarray(a, dtype=np.float32)).astype(ml_dtypes.bfloat16)


def _build_program(weights):
    """weights: dict with w31,w32,w41,w42,b31,b32,b41,b42 as f32 np arrays."""
    import concourse.mybir as mybir
    import concourse.tile as tile
    from concourse import bacc
    from concourse.masks import make_identity

    dt = mybir.dt
    AF = mybir.ActivationFunctionType
    ALU = mybir.AluOpType
    bf = dt.bfloat16
    f32 = dt.float32

    B = B_PER_CORE
    RCHUNKS = [(0, 512), (512, 512), (1024, 256)]
    BGROUPS = [(0, 6), (6, 6), (12, 4)]

    nc = bacc.Bacc("TRN2", target_bir_lowering=False, debug=False)

    i8 = dt.int8
    x3_d = nc.dram_tensor("x3", [B, NN, 1024], i8, kind="ExternalInput").ap()
    x4_d = nc.dram_tensor("x4", [B, NN, 2048], i8, kind="ExternalInput").ap()
    s3_d = nc.dram_tensor("s3", [NN, B], f32, kind="ExternalInput").ap()
    s4_d = nc.dram_tensor("s4", [NN, B], f32, kind="ExternalInput").ap()
    H_d = nc.dram_tensor("H", [NN, NN], f32, kind="ExternalInput").ap()
    w31_d = nc.inline_tensor(_bf16(weights["w31"]), name="w31c").ap()
    w32_d = nc.inline_tensor(_bf16(weights["w32"]), name="w32c").ap()
    w41_d = nc.inline_tensor(_bf16(weights["w41"]), name="w41c").ap()
    w42_d = nc.inline_tensor(_bf16(weights["w42"]), name="w42c").ap()
    b31_d = nc.inline_tensor(_bf16(weights["b31"]).reshape(1, -1), name="b31c").ap()
    b32_d = nc.inline_tensor(_bf16(weights["b32"]).reshape(1, -1), name="b32c").ap()
    b41_d = nc.inline_tensor(_bf16(weights["b41"]).reshape(1, -1), name="b41c").ap()
    b42_d = nc.inline_tensor(_bf16(weights["b42"]).reshape(1, -1), name="b42c").ap()
    out_d = nc.dram_tensor("out", [B, NN, 3072], f32, kind="ExternalOutput").ap()

    with tile.TileContext(nc) as tc:
        with tc.tile_pool(name="const", bufs=1) as cpool:
            G_r = cpool.tile([NN, NN], bf)
            GP_SHIFTS = [0, 16, 32, 48, 64, 80, 96, 112, -16, -32, -48, -64]
            gpad = {}
            for s in GP_SHIFTS:
                gpad[s] = cpool.tile([128, NN], bf, tag=f"gpad{s}", name=f"gpad{s}")
            grow_r = cpool.tile([1, R], bf)
            ones128_r = cpool.tile([1, 128], bf)

            # ---- G setup (tiny, fp32) ----
            with tc.tile_pool(name="gsetup", bufs=1) as gp, \
                 tc.tile_pool(name="gps", bufs=1, space="PSUM") as gpsum:
                ident = gp.tile([NN, NN], f32)
                make_identity(nc, ident[:])
                ones_col = gp.tile([NN, 1], f32)
                nc.vector.memset(ones_col[:], 1.0)
                Hsb = gp.tile([NN, NN], f32)
                nc.sync.dma_start(Hsb[:], H_d)
                Hs = gp.tile([NN, NN], f32)
                nc.scalar.activation(Hs[:], Hsb[:], AF.Sigmoid)
                dv = gp.tile([NN, 1], f32)
                nc.vector.tensor_reduce(dv[:], Hs[:], mybir.AxisListType.X, ALU.add)
                sq = gp.tile([NN, 1], f32)
                nc.scalar.sqrt(sq[:], dv[:])
                dv2 = gp.tile([NN, 1], f32)
                nc.vector.reciprocal(dv2[:], sq[:])
                Hp = gp.tile([NN, NN], f32)
                nc.scalar.mul(Hp[:], Hs[:], dv2[:])  # Hs * dv2[n]
                ps_de = gpsum.tile([NN, 1], f32)
                nc.tensor.matmul(ps_de[:], Hs[:], ones_col[:], start=True, stop=True)
                inv_de = gp.tile([NN, 1], f32)
                nc.vector.reciprocal(inv_de[:], ps_de[:])
                ps_hpt = gpsum.tile([NN, NN], f32)
                nc.tensor.matmul(ps_hpt[:], Hp[:], ident[:], start=True, stop=True)
                HpT = gp.tile([NN, NN], f32)
                nc.vector.tensor_copy(out=HpT[:], in_=ps_hpt[:])
                HpTs = gp.tile([NN, NN], f32)
                nc.scalar.mul(HpTs[:], ps_hpt[:], inv_de[:])  # HpT * inv_de[e]
                ps_G = gpsum.tile([NN, NN], f32)
                nc.tensor.matmul(ps_G[:], HpTs[:], HpT[:], start=True, stop=True)
                nc.vector.tensor_copy(out=G_r[:], in_=ps_G[:])
                G32 = gp.tile([NN, NN], f32)
                nc.scalar.copy(G32[:], ps_G[:])
                for s in GP_SHIFTS:
                    sel = gp.tile([NN, 128], f32, tag="sel")
                    nc.gpsimd.memset(sel[:], 0.0)
                    nc.gpsimd.affine_select(
                        out=sel[:], in_=sel[:],
                        compare_op=ALU.not_equal, fill=1.0,
                        base=s, pattern=[[-1, 128]], channel_multiplier=1)
                    ps_sel = gpsum.tile([128, NN], f32, tag="ps_sel")
                    nc.tensor.matmul(ps_sel[:], sel[:], G32[:], start=True, stop=True)
                    nc.vector.tensor_copy(out=gpad[s][:], in_=ps_sel[:])
                ps_g = gpsum.tile([NN, 1], f32)
                nc.tensor.matmul(ps_g[:], G32[:], ones_col[:], start=True, stop=True)
                g_col = gp.tile([NN, 1], f32)
                nc.vector.tensor_copy(out=g_col[:], in_=ps_g[:])
                ps_gr = gpsum.tile([1, NN], f32)
                nc.tensor.matmul(ps_gr[:], g_col[:], ident[:], start=True, stop=True)
                g_row = gp.tile([1, NN], f32)
                nc.vector.tensor_copy(out=g_row[:], in_=ps_gr[:])
                for b in range(B):
                    nc.vector.tensor_copy(out=grow_r[:, b * NN:(b + 1) * NN], in_=g_row[:])
                nc.vector.memset(ones128_r[:], 1.0)

            def build_stage(x_d, s_d, w1_d, b1_d, w2_d, b2_d, col_off, D):
                KT = D // 128
                DC = D // 512
                # non-LIFO pool lifetimes (queue alloc mode):
                #   biasp, afm | xp,psA (phase A) | hfm, wp,psB (phase B) |
                #   free afm | w2p,yz,psY,psZ (phase C)
                biasp_cm = tc.tile_pool(name=f"bias{D}", bufs=1)
                biasp = biasp_cm.__enter__()
                b1_s = biasp.tile([1, D], bf)
                b2_s = biasp.tile([1, D], bf)
                s_sb = biasp.tile([NN, B], f32)
                nc.sync.dma_start(b1_s[:], b1_d)
                nc.sync.dma_start(b2_s[:], b2_d)
                nc.sync.dma_start(s_sb[:], s_d)
                afm_cm = tc.tile_pool(name=f"afm{D}", bufs=1, side="right")
                afm_pool = afm_cm.__enter__()
                A_fm = afm_pool.tile([128, KT, R], bf)
                # phase A: AGG-B (RM -> FM); x arrives int8, dequant on ACT
                with tc.tile_pool(name=f"xqp{D}", bufs=2) as xqpool, \
                     tc.tile_pool(name=f"xp{D}", bufs=2) as xpool, \
                     tc.tile_pool(name=f"psA{D}", bufs=2, space="PSUM") as psumA:
                    for (b0, blen) in BGROUPS:
                        xq = xqpool.tile([NN, 6, D], i8, tag="xq")
                        xg = xpool.tile([NN, 6, D], bf, tag="xg")
                        for j in range(blen):
                            nc.sync.dma_start(xq[:, j], x_d[b0 + j])
                            nc.scalar.mul(xg[:, j], xq[:, j],
                                          s_sb[:, b0 + j:b0 + j + 1])
                        for kt in range(KT):
                            psA = psumA.tile([128, 6 * NN], f32)
                            for j in range(blen):
                                nc.tensor.matmul(
                                    psA[:, j * NN:(j + 1) * NN],
                                    xg[:, j, kt * 128:(kt + 1) * 128],
                                    G_r[:],
                                    start=True, stop=True)
                            nc.vector.tensor_copy(
                                out=A_fm[:, kt, b0 * NN:(b0 + blen) * NN],
                                in_=psA[:, :blen * NN])
                hfm_cm = tc.tile_pool(name=f"hfm{D}", bufs=1)
                hfm_pool = hfm_cm.__enter__()
                H_fm = hfm_pool.tile([128, KT, R], bf)
                # phase B: MUL-A + bias + relu (FM -> FM)
                with tc.tile_pool(name=f"wp{D}", bufs=2) as wpool, \
                     tc.tile_pool(name=f"psB{D}", bufs=4, space="PSUM") as psumB:
                    for dto in range(KT):
                        w1t = wpool.tile([128, KT, 128], bf, tag="w1t")
                        for kt in range(KT):
                            nc.sync.dma_start(
                                w1t[:, kt],
                                w1_d[kt * 128:(kt + 1) * 128,
                                     dto * 128:(dto + 1) * 128])
                        for (r0, rl) in RCHUNKS:
                            ps = psumB.tile([128, 512], f32)
                            for kt in range(KT):
                                nc.tensor.matmul(
                                    ps[:, :rl], w1t[:, kt],
                                    A_fm[:, kt, r0:r0 + rl],
                                    start=(kt == 0), stop=False)
                            nc.tensor.matmul(
                                ps[:, :rl],
                                b1_s[:, dto * 128:(dto + 1) * 128],
                                grow_r[:, r0:r0 + rl],
                                start=False, stop=True)
                            nc.scalar.activation(
                                H_fm[:, dto, r0:r0 + rl], ps[:, :rl], AF.Relu)
                afm_cm.__exit__(None, None, None)
                # phase C: MUL-B dense (M=128 r-rows), bias, AGG-A, DMA out.
                # 1280 r-rows = 10 dense tiles of 128; batches not crossing a
                # 128-row boundary feed AGG-A via base-partition slices, the
                # rest are assembled with tiny SBUF->SBUF partition-shift DMAs.
                NT = R // 128  # 10
                with tc.tile_pool(name=f"w2p{D}", bufs=2) as w2pool, \
                     tc.tile_pool(name=f"yd{D}", bufs=NT + 1) as ydpool, \
                     tc.tile_pool(name=f"yz{D}", bufs=3) as yzpool, \
                     tc.tile_pool(name=f"psY{D}", bufs=2, space="PSUM") as psumY, \
                     tc.tile_pool(name=f"psZ{D}", bufs=2, space="PSUM") as psumZ:
                    for dc in range(DC):
                        w2c = w2pool.tile([128, KT, 512], bf, tag="w2c")
                        for kt in range(KT):
                            nc.sync.dma_start(
                                w2c[:, kt],
                                w2_d[kt * 128:(kt + 1) * 128,
                                     dc * 512:(dc + 1) * 512])
                        dense = []
                        for t in range(NT):
                            psy = psumY.tile([128, 512], f32)
                            for kt in range(KT):
                                nc.tensor.matmul(
                                    psy[:], H_fm[:, kt, t * 128:(t + 1) * 128],
                                    w2c[:, kt], start=(kt == 0), stop=False)
                            nc.tensor.matmul(
                                psy[:], ones128_r[:],
                                b2_s[:, dc * 512:(dc + 1) * 512],
                                start=False, stop=True)
                            ydn = ydpool.tile([128, 512], bf, tag="yd")
                            nc.vector.tensor_copy(out=ydn[:], in_=psy[:])
                            dense.append(ydn)
                        for b in range(B):
                            r0 = b * NN
                            t0, o0 = divmod(r0, 128)
                            psz = psumZ.tile([NN, 512], f32)
                            if o0 <= 48:
                                nc.tensor.matmul(psz[:], gpad[o0][:], dense[t0][:],
                                                 start=True, stop=True)
                            else:
                                nc.tensor.matmul(psz[:], gpad[o0][:], dense[t0][:],
                                                 start=True, stop=False)
                                nc.tensor.matmul(psz[:], gpad[o0 - 128][:], dense[t0 + 1][:],
                                                 start=False, stop=True)
                            zsb = yzpool.tile([NN, 512], f32, tag="z")
                            nc.scalar.copy(zsb[:], psz[:])
                            nc.sync.dma_start(
                                out_d[b, :, col_off + dc * 512:col_off + (dc + 1) * 512],
                                zsb[:])
                hfm_cm.__exit__(None, None, None)
                biasp_cm.__exit__(None, None, None)

            _only3 = os.environ.get("K_ONLY_STAGE3") == "1"
            build_stage(x3_d, s3_d, w31_d, b31_d, w32_d, b32_d, 0, 1024)
            if not _only3:
                build_stage(x4_d, s4_d, w41_d, b41_d, w42_d, b42_d, 1024, 2048)

    nc.compile()
    return nc


def _weights_key(inputs):
    import hashlib
    h = hashlib.sha256()
    for k in ("w31", "w32", "w41", "w42", "b31", "b32", "b41", "b42"):
        a = np.ascontiguousarray(np.asarray(inputs[k], dtype=np.float32))
        h.update(k.encode())
        h.update(str(a.shape).encode())
        h.update(a.tobytes())
    return h.hexdigest()


def _make_runner(nc):
    """Persistent jitted SPMD runner: fn(x3g, x4g, Hg) -> (out_global,).

    No zero-output donation: the kernel writes every output element, so XLA
    can allocate the custom-call results directly.
    """
    import jax
    import concourse.mybir as mybir
    from concourse import bass2jax
    from jax.sharding import Mesh, PartitionSpec
    try:
        from jax.experimental.shard_map import shard_map
    except ImportError:
        from jax.shard_map import shard_map

    bass2jax.install_neuronx_cc_hook()

    partition_name = nc.partition_id_tensor.name if nc.partition_id_tensor else None
    in_names, out_names, out_avals = [], [], []
    for alloc in nc.m.functions[0].allocations:
        if not isinstance(alloc, mybir.MemoryLocationSet):
            continue
        name = alloc.memorylocations[0].name
        if alloc.kind == "ExternalInput":
            if name != partition_name:
                in_names.append(name)
        elif alloc.kind == "ExternalOutput":
            out_names.append(name)
            shape = tuple(alloc.tensor_shape)
            dtype = mybir.dt.np(alloc.dtype)
            out_avals.append(jax.core.ShapedArray(shape, dtype))
    n_params = len(in_names)
    all_names = list(in_names)
    if partition_name is not None:
        all_names = all_names + [partition_name]

    def _body(*args):
        operands = list(args)
        if partition_name is not None:
            operands.append(bass2jax.partition_id_tensor())
        outs = bass2jax._bass_exec_p.bind(
            *operands,
            out_avals=tuple(out_avals),
            in_names=tuple(all_names),
            out_names=tuple(out_names),
            lowering_input_output_aliases=(),
            sim_require_finite=True,
            sim_require_nnan=True,
            nc=nc,
        )
        return tuple(outs)

    devices = jax.devices()[:N_CORES]
    mesh = Mesh(np.asarray(devices), ("core",))
    n_out = len(out_names)
    fn = jax.jit(shard_map(
        _body, mesh=mesh,
        in_specs=(PartitionSpec("core"),) * n_params,
        out_specs=(PartitionSpec("core"),) * n_out,
        check_rep=False,
    ))
    return fn, in_names, out_names


def _quant8(x):
    """Per-(batch,node) int8 quantization. x: [B,N,D] f32 -> (q int8, s f32[B,N])."""
    x = np.ascontiguousarray(np.asarray(x, dtype=np.float32))
    s = np.abs(x).max(axis=-1) / 127.0
    s = np.maximum(s, 1e-30).astype(np.float32)
    q = np.clip(np.rint(x / s[..., None]), -127, 127).astype(np.int8)
    return q, s


def _scales_global(s):
    """s: [128, 80] f32 -> global [8*80, 16] (per-core [NN, B] transposed)."""
    return np.ascontiguousarray(
        s.reshape(N_CORES, B_PER_CORE, NN).transpose(0, 2, 1).reshape(
            N_CORES * NN, B_PER_CORE))


def _global_inputs(inputs):
    """Full-size (unsharded) runner inputs, keyed by DRAM tensor name."""
    x3q, s3 = _quant8(inputs["stage_3_input"])    # [128,80,1024] i8, [128,80]
    x4q, s4 = _quant8(inputs["input_x"])          # [128,80,2048] i8, [128,80]
    Hf = np.ascontiguousarray(np.asarray(inputs["H"], dtype=np.float32))
    Hg = np.broadcast_to(Hf, (N_CORES,) + Hf.shape).reshape(N_CORES * NN, NN)
    Hg = np.ascontiguousarray(Hg)                 # [8*80, 80] f32 (replicated)
    return {"x3": x3q, "x4": x4q,
            "s3": _scales_global(s3), "s4": _scales_global(s4), "H": Hg}


def get_state(inputs):
    key = _weights_key(inputs)
    if key not in _CACHE:
        nc = _build_program({k: inputs[k] for k in
                             ("w31", "w32", "w41", "w42",
                              "b31", "b32", "b41", "b42")})
        fn, in_names, out_names = _make_runner(nc)
        _CACHE[key] = {"nc": nc, "fn": fn, "in_names": in_names,
                       "out_names": out_names}
    return _CACHE[key]


def kernel(**inputs):
    import jax
    st = get_state(inputs)
    gi = _global_inputs(inputs)
    args = [gi[n] for n in st["in_names"]]
    outs = jax.block_until_ready(st["fn"](*args))
    out = np.asarray(outs[0]).reshape(128, NN, 3072)
    return np.ascontiguousarray(out.astype(np.float32))


# revision 16
# speedup vs baseline: 1.4063x; 1.4063x over previous
"""HGNN (2-stage hypergraph conv) kernel for Trainium2.

Data-parallel over batch across 8 NeuronCores (16 batches/core). Weights and
biases are embedded in the NEFF as Const tensors (cast to bf16) so they are
DMA'd to HBM once at model-load time; per-iteration traffic is only the
activations (bf16) in and the f32 output out. Matmuls run on the PE in bf16
with f32 PSUM accumulation.

Per-core plan (stage = conv(conv(x))):
  G setup     : G = DV^-1/2 Hs DE^-1 Hs^T DV^-1/2 computed on-device (tiny, fp32).
                G is symmetric. g = G @ 1 for the aggregated-bias term.
  phase A     : A_fm[d,(b,m)] = (G X_b)^T   -- AGG-B: activation-stationary
                matmuls (lhsT=X_b[80,128-dtile], rhs=G) -> RM->FM "free" transpose.
  phase B     : H_fm = relu(A_fm.T W1 + g (x) b1) -- weight-stationary matmuls
                accumulating over din tiles + a K=1 bias-row matmul; ACT relu
                copyback straight from PSUM (FM->FM).
  phase C     : per (dout-chunk, batch): Y = H_b^T W2 + b2 (activation-stationary,
                FM->RM), then Z = G Y (G-stationary), DMA out.
"""
import os
import numpy as np

_CACHE = {}

B_PER_CORE = 16
NN = 80
R = B_PER_CORE * NN  # 1280
N_CORES = 8


def _bf16(a):
    import ml_dtypes
    return np.ascontiguousarray(np.asarray(a, dtype=np.float32)).astype(ml_dtypes.bfloat16)


def _build_program(weights):
    """weights: dict with w31,w32,w41,w42,b31,b32,b41,b42 as f32 np arrays."""
    import concourse.mybir as mybir
    import concourse.tile as tile
    from concourse import bacc
    from concourse.masks import make_identity

    dt = mybir.dt
    AF = mybir.ActivationFunctionType
    ALU = mybir.AluOpType
    bf = dt.bfloat16
    f32 = dt.float32

    B = B_PER_CORE
    RCHUNKS = [(0, 512), (512, 512), (1024, 256)]
    BGROUPS = [(0, 6), (6, 6), (12, 4)]

    nc = bacc.Bacc("TRN2", target_bir_lowering=False, debug=False)

    i8 = dt.int8
    # xin row r = (b*NN+n): [ x3[b,n,:] int8 | x4[b,n,:] int8 ]
    # meta row n: [ s3[n, 0:16] | s4[n, 0:16] | H[n, 0:80] ] f32
    xin_d = nc.dram_tensor("xin", [R, 3072], i8, kind="ExternalInput").ap()
    meta_d = nc.dram_tensor("meta", [NN, 112], f32, kind="ExternalInput").ap()
    w31_d = nc.inline_tensor(_bf16(weights["w31"]), name="w31c").ap()
    w32_d = nc.inline_tensor(_bf16(weights["w32"]), name="w32c").ap()
    w41_d = nc.inline_tensor(_bf16(weights["w41"]), name="w41c").ap()
    w42_d = nc.inline_tensor(_bf16(weights["w42"]), name="w42c").ap()
    b31_d = nc.inline_tensor(_bf16(weights["b31"]).reshape(1, -1), name="b31c").ap()
    b32_d = nc.inline_tensor(_bf16(weights["b32"]).reshape(1, -1), name="b32c").ap()
    b41_d = nc.inline_tensor(_bf16(weights["b41"]).reshape(1, -1), name="b41c").ap()
    b42_d = nc.inline_tensor(_bf16(weights["b42"]).reshape(1, -1), name="b42c").ap()
    out_d = nc.dram_tensor("out", [B, NN, 3072], f32, kind="ExternalOutput").ap()

    with tile.TileContext(nc) as tc:
        with tc.tile_pool(name="const", bufs=1) as cpool:
            G_r = cpool.tile([NN, NN], bf)
            GP_SHIFTS = [0, 16, 32, 48, 64, 80, 96, 112, -16, -32, -48, -64]
            gpad = {}
            for s in GP_SHIFTS:
                gpad[s] = cpool.tile([128, NN], bf, tag=f"gpad{s}", name=f"gpad{s}")
            grow_r = cpool.tile([1, R], bf)
            ones128_r = cpool.tile([1, 128], bf)
            meta_sb = cpool.tile([NN, 112], f32)
            nc.sync.dma_start(meta_sb[:], meta_d)

            # ---- G setup (tiny, fp32) ----
            with tc.tile_pool(name="gsetup", bufs=1) as gp, \
                 tc.tile_pool(name="gps", bufs=1, space="PSUM") as gpsum:
                ident = gp.tile([NN, NN], f32)
                make_identity(nc, ident[:])
                ones_col = gp.tile([NN, 1], f32)
                nc.vector.memset(ones_col[:], 1.0)
                Hs = gp.tile([NN, NN], f32)
                nc.scalar.activation(Hs[:], meta_sb[:, 32:112], AF.Sigmoid)
                dv = gp.tile([NN, 1], f32)
                nc.vector.tensor_reduce(dv[:], Hs[:], mybir.AxisListType.X, ALU.add)
                sq = gp.tile([NN, 1], f32)
                nc.scalar.sqrt(sq[:], dv[:])
                dv2 = gp.tile([NN, 1], f32)
                nc.vector.reciprocal(dv2[:], sq[:])
                Hp = gp.tile([NN, NN], f32)
                nc.scalar.mul(Hp[:], Hs[:], dv2[:])  # Hs * dv2[n]
                ps_de = gpsum.tile([NN, 1], f32)
                nc.tensor.matmul(ps_de[:], Hs[:], ones_col[:], start=True, stop=True)
                inv_de = gp.tile([NN, 1], f32)
                nc.vector.reciprocal(inv_de[:], ps_de[:])
                ps_hpt = gpsum.tile([NN, NN], f32)
                nc.tensor.matmul(ps_hpt[:], Hp[:], ident[:], start=True, stop=True)
                HpT = gp.tile([NN, NN], f32)
                nc.vector.tensor_copy(out=HpT[:], in_=ps_hpt[:])
                HpTs = gp.tile([NN, NN], f32)
                nc.scalar.mul(HpTs[:], ps_hpt[:], inv_de[:])  # HpT * inv_de[e]
                ps_G = gpsum.tile([NN, NN], f32)
                nc.tensor.matmul(ps_G[:], HpTs[:], HpT[:], start=True, stop=True)
                nc.vector.tensor_copy(out=G_r[:], in_=ps_G[:])
                G32 = gp.tile([NN, NN], f32)
                nc.scalar.copy(G32[:], ps_G[:])
                for s in GP_SHIFTS:
                    sel = gp.tile([NN, 128], f32, tag="sel")
                    nc.gpsimd.memset(sel[:], 0.0)
                    nc.gpsimd.affine_select(
                        out=sel[:], in_=sel[:],
                        compare_op=ALU.not_equal, fill=1.0,
                        base=s, pattern=[[-1, 128]], channel_multiplier=1)
                    ps_sel = gpsum.tile([128, NN], f32, tag="ps_sel")
                    nc.tensor.matmul(ps_sel[:], sel[:], G32[:], start=True, stop=True)
                    nc.vector.tensor_copy(out=gpad[s][:], in_=ps_sel[:])
                ps_g = gpsum.tile([NN, 1], f32)
                nc.tensor.matmul(ps_g[:], G32[:], ones_col[:], start=True, stop=True)
                g_col = gp.tile([NN, 1], f32)
                nc.vector.tensor_copy(out=g_col[:], in_=ps_g[:])
                ps_gr = gpsum.tile([1, NN], f32)
                nc.tensor.matmul(ps_gr[:], g_col[:], ident[:], start=True, stop=True)
                g_row = gp.tile([1, NN], f32)
                nc.vector.tensor_copy(out=g_row[:], in_=ps_gr[:])
                for b in range(B):
                    nc.vector.tensor_copy(out=grow_r[:, b * NN:(b + 1) * NN], in_=g_row[:])
                nc.vector.memset(ones128_r[:], 1.0)

            def build_stage(xoff, soff, w1_d, b1_d, w2_d, b2_d, col_off, D):
                KT = D // 128
                DC = D // 512
                # non-LIFO pool lifetimes (queue alloc mode):
                #   biasp, afm | xp,psA (phase A) | hfm, wp,psB (phase B) |
                #   free afm | w2p,yz,psY,psZ (phase C)
                biasp_cm = tc.tile_pool(name=f"bias{D}", bufs=1)
                biasp = biasp_cm.__enter__()
                b1_s = biasp.tile([1, D], bf)
                b2_s = biasp.tile([1, D], bf)
                nc.sync.dma_start(b1_s[:], b1_d)
                nc.sync.dma_start(b2_s[:], b2_d)
                afm_cm = tc.tile_pool(name=f"afm{D}", bufs=1, side="right")
                afm_pool = afm_cm.__enter__()
                A_fm = afm_pool.tile([128, KT, R], bf)
                # phase A: AGG-B (RM -> FM); x arrives int8, dequant on ACT
                with tc.tile_pool(name=f"xqp{D}", bufs=2) as xqpool, \
                     tc.tile_pool(name=f"xp{D}", bufs=2) as xpool, \
                     tc.tile_pool(name=f"psA{D}", bufs=2, space="PSUM") as psumA:
                    for (b0, blen) in BGROUPS:
                        xq = xqpool.tile([NN, 6, D], i8, tag="xq")
                        xg = xpool.tile([NN, 6, D], bf, tag="xg")
                        for j in range(blen):
                            nc.sync.dma_start(
                                xq[:, j],
                                xin_d[(b0 + j) * NN:(b0 + j + 1) * NN,
                                      xoff:xoff + D])
                            nc.scalar.mul(xg[:, j], xq[:, j],
                                          meta_sb[:, soff + b0 + j:
                                                  soff + b0 + j + 1])
                        for kt in range(KT):
                            psA = psumA.tile([128, 6 * NN], f32)
                            for j in range(blen):
                                nc.tensor.matmul(
                                    psA[:, j * NN:(j + 1) * NN],
                                    xg[:, j, kt * 128:(kt + 1) * 128],
                                    G_r[:],
                                    start=True, stop=True)
                            nc.vector.tensor_copy(
                                out=A_fm[:, kt, b0 * NN:(b0 + blen) * NN],
                                in_=psA[:, :blen * NN])
                hfm_cm = tc.tile_pool(name=f"hfm{D}", bufs=1)
                hfm_pool = hfm_cm.__enter__()
                H_fm = hfm_pool.tile([128, KT, R], bf)
                # phase B: MUL-A + bias + relu (FM -> FM)
                with tc.tile_pool(name=f"wp{D}", bufs=2) as wpool, \
                     tc.tile_pool(name=f"psB{D}", bufs=4, space="PSUM") as psumB:
                    for dto in range(KT):
                        w1t = wpool.tile([128, KT, 128], bf, tag="w1t")
                        for kt in range(KT):
                            nc.sync.dma_start(
                                w1t[:, kt],
                                w1_d[kt * 128:(kt + 1) * 128,
                                     dto * 128:(dto + 1) * 128])
                        for (r0, rl) in RCHUNKS:
                            ps = psumB.tile([128, 512], f32)
                            for kt in range(KT):
                                nc.tensor.matmul(
                                    ps[:, :rl], w1t[:, kt],
                                    A_fm[:, kt, r0:r0 + rl],
                                    start=(kt == 0), stop=False)
                            nc.tensor.matmul(
                                ps[:, :rl],
                                b1_s[:, dto * 128:(dto + 1) * 128],
                                grow_r[:, r0:r0 + rl],
                                start=False, stop=True)
                            nc.scalar.activation(
                                H_fm[:, dto, r0:r0 + rl], ps[:, :rl], AF.Relu)
                afm_cm.__exit__(None, None, None)
                # phase C: MUL-B dense (M=128 r-rows), bias, AGG-A, DMA out.
                # 1280 r-rows = 10 dense tiles of 128; batches not crossing a
                # 128-row boundary feed AGG-A via base-partition slices, the
                # rest are assembled with tiny SBUF->SBUF partition-shift DMAs.
                NT = R // 128  # 10
                with tc.tile_pool(name=f"w2p{D}", bufs=2) as w2pool, \
                     tc.tile_pool(name=f"yd{D}", bufs=NT + 1) as ydpool, \
                     tc.tile_pool(name=f"yz{D}", bufs=3) as yzpool, \
                     tc.tile_pool(name=f"psY{D}", bufs=2, space="PSUM") as psumY, \
                     tc.tile_pool(name=f"psZ{D}", bufs=2, space="PSUM") as psumZ:
                    for dc in range(DC):
                        w2c = w2pool.tile([128, KT, 512], bf, tag="w2c")
                        for kt in range(KT):
                            nc.sync.dma_start(
                                w2c[:, kt],
                                w2_d[kt * 128:(kt + 1) * 128,
                                     dc * 512:(dc + 1) * 512])
                        dense = []
                        for t in range(NT):
                            psy = psumY.tile([128, 512], f32)
                            for kt in range(KT):
                                nc.tensor.matmul(
                                    psy[:], H_fm[:, kt, t * 128:(t + 1) * 128],
                                    w2c[:, kt], start=(kt == 0), stop=False)
                            nc.tensor.matmul(
                                psy[:], ones128_r[:],
                                b2_s[:, dc * 512:(dc + 1) * 512],
                                start=False, stop=True)
                            ydn = ydpool.tile([128, 512], bf, tag="yd")
                            nc.vector.tensor_copy(out=ydn[:], in_=psy[:])
                            dense.append(ydn)
                        for b in range(B):
                            r0 = b * NN
                            t0, o0 = divmod(r0, 128)
                            psz = psumZ.tile([NN, 512], f32)
                            if o0 <= 48:
                                nc.tensor.matmul(psz[:], gpad[o0][:], dense[t0][:],
                                                 start=True, stop=True)
                            else:
                                nc.tensor.matmul(psz[:], gpad[o0][:], dense[t0][:],
                                                 start=True, stop=False)
                                nc.tensor.matmul(psz[:], gpad[o0 - 128][:], dense[t0 + 1][:],
                                                 start=False, stop=True)
                            zsb = yzpool.tile([NN, 512], f32, tag="z")
                            nc.scalar.copy(zsb[:], psz[:])
                            nc.sync.dma_start(
                                out_d[b, :, col_off + dc * 512:col_off + (dc + 1) * 512],
                                zsb[:])
                hfm_cm.__exit__(None, None, None)
                biasp_cm.__exit__(None, None, None)

            _only3 = os.environ.get("K_ONLY_STAGE3") == "1"
            build_stage(0, 0, w31_d, b31_d, w32_d, b32_d, 0, 1024)
            if not _only3:
                build_stage(1024, 16, w41_d, b41_d, w42_d, b42_d, 1024, 2048)

    nc.compile()
    return nc


def _weights_key(inputs):
    import hashlib
    h = hashlib.sha256()
    for k in ("w31", "w32", "w41", "w42", "b31", "b32", "b41", "b42"):
        a = np.ascontiguousarray(np.asarray(inputs[k], dtype=np.float32))
        h.update(k.encode())
        h.update(str(a.shape).encode())
        h.update(a.tobytes())
    return h.hexdigest()


def _make_runner(nc):
    """Persistent jitted SPMD runner: fn(x3g, x4g, Hg) -> (out_global,).

    No zero-output donation: the kernel writes every output element, so XLA
    can allocate the custom-call results directly.
    """
    import jax
    import concourse.mybir as mybir
    from concourse import bass2jax
    from jax.sharding import Mesh, PartitionSpec
    try:
        from jax.experimental.shard_map import shard_map
    except ImportError:
        from jax.shard_map import shard_map

    bass2jax.install_neuronx_cc_hook()

    partition_name = nc.partition_id_tensor.name if nc.partition_id_tensor else None
    in_names, out_names, out_avals = [], [], []
    for alloc in nc.m.functions[0].allocations:
        if not isinstance(alloc, mybir.MemoryLocationSet):
            continue
        name = alloc.memorylocations[0].name
        if alloc.kind == "ExternalInput":
            if name != partition_name:
                in_names.append(name)
        elif alloc.kind == "ExternalOutput":
            out_names.append(name)
            shape = tuple(alloc.tensor_shape)
            dtype = mybir.dt.np(alloc.dtype)
            out_avals.append(jax.core.ShapedArray(shape, dtype))
    n_params = len(in_names)
    all_names = list(in_names)
    if partition_name is not None:
        all_names = all_names + [partition_name]

    def _body(*args):
        operands = list(args)
        if partition_name is not None:
            operands.append(bass2jax.partition_id_tensor())
        outs = bass2jax._bass_exec_p.bind(
            *operands,
            out_avals=tuple(out_avals),
            in_names=tuple(all_names),
            out_names=tuple(out_names),
            lowering_input_output_aliases=(),
            sim_require_finite=True,
            sim_require_nnan=True,
            nc=nc,
        )
        return tuple(outs)

    devices = jax.devices()[:N_CORES]
    mesh = Mesh(np.asarray(devices), ("core",))
    n_out = len(out_names)
    fn = jax.jit(shard_map(
        _body, mesh=mesh,
        in_specs=(PartitionSpec("core"),) * n_params,
        out_specs=(PartitionSpec("core"),) * n_out,
        check_rep=False,
    ))
    return fn, in_names, out_names


def _quant8(x):
    """Per-(batch,node) int8 quantization. x: [B,N,D] f32 -> (q int8, s f32[B,N])."""
    x = np.ascontiguousarray(np.asarray(x, dtype=np.float32))
    s = np.abs(x).max(axis=-1) / 127.0
    s = np.maximum(s, 1e-30).astype(np.float32)
    q = np.clip(np.rint(x / s[..., None]), -127, 127).astype(np.int8)
    return q, s


def _scales_global(s):
    """s: [128, 80] f32 -> global [8*80, 16] (per-core [NN, B] transposed)."""
    return np.ascontiguousarray(
        s.reshape(N_CORES, B_PER_CORE, NN).transpose(0, 2, 1).reshape(
            N_CORES * NN, B_PER_CORE))


def _global_inputs(inputs):
    """Full-size (unsharded) runner inputs, keyed by DRAM tensor name."""
    x3q, s3 = _quant8(inputs["stage_3_input"])    # [128,80,1024] i8, [128,80]
    x4q, s4 = _quant8(inputs["input_x"])          # [128,80,2048] i8, [128,80]
    # xin: [8*1280, 3072] int8, row (core,b,n) = [x3 row | x4 row]
    xin = np.concatenate(
        [x3q.reshape(128 * NN, 1024), x4q.reshape(128 * NN, 2048)], axis=1)
    xin = np.ascontiguousarray(xin)
    # meta: [8*80, 112] f32, per-core rows n = [s3[:,b] | s4[:,b] | H[n,:]]
    Hf = np.ascontiguousarray(np.asarray(inputs["H"], dtype=np.float32))
    Hrep = np.broadcast_to(Hf, (N_CORES,) + Hf.shape)  # [8,80,80]
    meta = np.concatenate(
        [_scales_global(s3).reshape(N_CORES, NN, B_PER_CORE),
         _scales_global(s4).reshape(N_CORES, NN, B_PER_CORE),
         Hrep], axis=2).reshape(N_CORES * NN, 112)
    meta = np.ascontiguousarray(meta)
    return {"xin": xin, "meta": meta}


def get_state(inputs):
    key = _weights_key(inputs)
    if key not in _CACHE:
        nc = _build_program({k: inputs[k] for k in
                             ("w31", "w32", "w41", "w42",
                              "b31", "b32", "b41", "b42")})
        fn, in_names, out_names = _make_runner(nc)
        _CACHE[key] = {"nc": nc, "fn": fn, "in_names": in_names,
                       "out_names": out_names}
    return _CACHE[key]


def kernel(**inputs):
    import jax
    st = get_state(inputs)
    gi = _global_inputs(inputs)
    args = [gi[n] for n in st["in_names"]]
    outs = jax.block_until_ready(st["fn"](*args))
    out = np.asarray(outs[0]).reshape(128, NN, 3072)
    return np.ascontiguousarray(out.astype(np.float32))
